# revision 51
# baseline (speedup 1.0000x reference)
"""Bidirectional LSTM (B=32, T=2048, I=256, H=128/dir) for 8 Trainium2 cores.

Strategy: data-parallel over (batch, direction) — cores 0-3 run the forward
LSTM over 8 batch rows each, cores 4-7 run the backward LSTM over the
host-flipped sequences.  Inside each core the nonlinear recurrence is solved
with block Gauss-Seidel fixed-point iteration: time is processed in blocks of
L steps; within a block, K sweeps each recompute all gates with one batched
matmul (accumulating W_hh @ delta_h into PSUM on top of the input-gate
precomputation), apply sigmoid/tanh over the whole block at once, run the
c-recurrence with the hardware tensor_tensor_scan, and recompute h.  The
iteration contracts by ~5-30x per sweep; the feedback path runs in fp16 (all
values bounded by 1).

Everything is gate-major: SBUF/PSUM tiles are [128 partitions = gate/h
element, cols = (batch-unit, time) b-major].

v3 structure (vs the original): K_SWEEPS=3 (rel-l2 ~5.4e-3, comfortably
under the 2e-2 gate); the cell state is kept halved (c' = c/2) so
z' = (sig(2g) - 0.5)*sig(i) is one fused scalar_tensor_tensor op and
tanh(c) = tanh(2c') uses the activation's free input scale; h lives in
[carry-slot | h] tiles whose col 0 is permanently zero (aligned full-range
PSUM matmul writes, no per-block memsets/copies), with the carry
contribution delivered once per block by a tiny strided matmul into the
(u, 0) gate columns (sweep-0 then needs no feedback matmul at all);
carries are produced from column L-1 directly so the next
block's sweep 0 does not wait for the full out multiply; sweep emission is
round-robin interleaved across streams so the in-order ACT/DVE queues
overlap them.  S=4 streams (C=256) models ~12% faster than S=2 (C=512):
the 4-way interleave keeps ScalarE ~95% busy, which beats the lower
fixed-cost-per-column of wider blocks.

Engine split: the two intermediate sweeps' tanh(2c') runs on the Vector
engine as a custom 8-stage DVE op (clamped odd quintic, ~1.9e-2 sup err —
the final sweep keeps the exact ScalarE tanh, so output precision is set
by the converged iterate); the intermediate h multiply and the final out
multiply run on the otherwise-idle Pool engine.  End-to-end rel-l2
8.45e-3 on HW vs the 2e-2 gate.  Modeled exec ~249us, ScalarE ~91% busy
on the irreducible 4C-sigmoid work (was ~505us at K=5 all-ScalarE).
Startup is overlapped: sigmoid-table preload via a dummy activation,
split weight DMAs, and DMA-independent dummy PE matmuls that complete
the p-state ramp inside the weight-DMA window.
Structural limits reached: C=256/S=4 is the PSUM ceiling (S*C <= 1024
f32 cols), K=3 the accuracy floor (K=2 measures ~2.4e-2 rel), and
offloading further sigmoid chunks to DVE costs ~2 DVE ops per ScalarE
chunk saved, which would overload DVE.
"""

import numpy as np

import concourse.bass as bass
import concourse.bacc as bacc
import concourse.tile as tile
from concourse import mybir
from concourse.bass_utils import run_bass_kernel_spmd

# --- custom DVE op: clamped odd-quintic tanh(2*c') ------------------------- #
# Intermediate sweeps' h feedback tolerates ~2e-2 absolute tanh error (the
# final sweep still uses the exact ScalarE tanh, so output precision is set
# there; end-to-end sim: rel-l2 5.2e-3 -> 8.3e-3).  Offloading these tanh
# instructions to the (underutilized) Vector engine takes them off the
# ScalarE critical resource.  8/8 v3 pipeline stages:
#   v = x*x; p = x*(C0 + v*(C1 + v*C2)); out = minn(maxx(p, Src1=-1), One)
import concourse.dve_ops as dve_ops
from concourse.dve_spec import Spec, Src0, Src1, C0, C1, C2, One, maxx, minn, lower
from concourse.dve_spec import _has_src1 as has_src1
from concourse.bass_utils import DveOpSpec

# minimax fit of tanh(2u) ~ clamp(u*(QA + QB u^2 + QC u^4), -1, 1); sup err 1.9e-2
QA, QB, QC = 1.8624705163395339, -1.4112853220773123, 0.4945361474462829


def _register_tanh2c_op():
    name = "TANH2C_APPROX_ANT"
    for op in dve_ops.OPS:
        if op.name == name:
            return op
    v = Src0 * Src0
    body = minn(maxx(Src0 * (C0 + v * (C1 + v * C2)), Src1), One)

    def _ref(in0, in1, s0, s1, imm2):
        vv = in0 * in0
        return np.minimum(
            np.maximum(in0 * (s0 + vv * (s1 + vv * imm2)), in1), 1.0)

    spec = Spec(body=body, reference=_ref)
    row = dve_ops._CUSTOM_DVE_ROW_BASE + len(dve_ops.OPS)
    assert row < 0x20, "custom DVE sub-opcode rows exhausted"
    dve_ops._SUB_OPCODE_FOR_NAME[name] = row
    shas = {}
    for ver in ("v3", "v4"):
        try:
            uops = lower(spec, ver=ver)
            shas[ver] = DveOpSpec(
                name=name, opcode=row, uops=uops,
                rd1_en=has_src1(spec)).sha(ver)
        except Exception:
            pass
    assert shas, "tanh2c spec failed to lower for every DVE version"
    op = dve_ops.DveOp(name, spec, subdim=False, uops_sha=shas)
    dve_ops.OPS.append(op)
    dve_ops.CUSTOM_DVE_SPECS[name] = spec
    return op


TANH2C_OP = _register_tanh2c_op()

# Problem shapes (hardcoded per contract)
B, T, I, HS = 32, 2048, 256, 256
H = 128          # per-direction hidden
G4 = 4 * H       # 512 stacked gates
NCORES = 8
U = 8            # sequences per core
S = 4            # independent streams per core (pipelining)
BS = U // S      # sequences per stream
L = 128          # time-block length
NBLK = T // L
K_SWEEPS = 3
C = BS * L       # columns per stream-block (512)

# engine-placement experiment knobs (read at _build_nc time)
HMUL_POOL = True    # non-last-sweep h = sig(o)*tanh(c) on Pool instead of DVE
Z_POOL = False      # z' on Pool is ILLEGAL: TensorScalarPtr is not a Pool
                    # opcode (walrus NCC_IXCG966); keep False
OMUL_POOL = True    # final out = sig(o)*tanh(c) on Pool instead of DVE
TANH_DVE = True     # non-last-sweep tanh via custom DVE op instead of ScalarE
SCAN_POOL = False   # run every other per-seq c-scan on Pool (parallel to DVE)

# gate chunk order inside the 4*H dim: (i, f, o, g); reference order is (i, f, g, o)
PERM = [0, 1, 3, 2]

F32 = mybir.dt.float32
BF16 = mybir.dt.bfloat16
F16 = mybir.dt.float16
F32R = mybir.dt.float32r

_NC_CACHE = {}


def _build_nc():
    nc = bacc.Bacc()
    xt_h = nc.dram_tensor("xt", [2, 128, U * T], F32R, kind="ExternalInput")
    wih_h = nc.dram_tensor("wih", [2, 128, G4], F32R, kind="ExternalInput")
    whh_h = nc.dram_tensor("whh", [128, G4], F16, kind="ExternalInput")
    bias_h = nc.dram_tensor("bias", [1, G4], F32R, kind="ExternalInput")
    out_h = nc.dram_tensor("out", [128, U * T], F32, kind="ExternalOutput")

    sig = mybir.ActivationFunctionType.Sigmoid
    tanh = mybir.ActivationFunctionType.Tanh
    mult = mybir.AluOpType.mult
    add = mybir.AluOpType.add

    with tile.TileContext(nc) as tc:
        with (
            tc.tile_pool(name="singles", bufs=1) as singles,
            tc.tile_pool(name="work", bufs=3) as work,
            tc.tile_pool(name="psum", bufs=1, space="PSUM") as psump,
        ):
            # --- constants / weights ---
            # Dummy activation first: triggers the sigmoid-table
            # ACT_TABLE_LOAD (~1.3us) while the weight DMAs and first xg
            # matmuls are still in flight, instead of serializing it before
            # the first real sigmoid.
            dummy = singles.tile([128, 1], F32, tag="dummy")
            nc.vector.memset(dummy, 0.0)
            nc.scalar.activation(out=dummy, in_=dummy,
                                 func=mybir.ActivationFunctionType.Sigmoid)
            # DMA-independent memsets first so they (and the dummy PE
            # warm-ups that consume ones_sb) never queue behind DMA-waiting
            # DVE work.
            ones_sb = singles.tile([1, C], F32R, tag="ones")
            nc.vector.memset(ones_sb.bitcast(mybir.dt.uint32), 0x3F800000)
            # full-width [-1] operand for the tanh clamp (a [P,1] broadcast
            # Src1 faults the DVE at the late maxx stage on hardware; the
            # full-width form is bit-exact)
            negone_sb = singles.tile([128, C], F32, tag="negone")
            nc.vector.memset(negone_sb, -1.0)
            wih_sb = singles.tile([128, 2, G4], F32R, tag="wih")
            # split per k-half so the k=0 xg matmuls can start while the
            # second half is still in flight
            for k in range(2):
                nc.sync.dma_start(out=wih_sb[:, k], in_=wih_h[k, :, :])
            whh_sb = singles.tile([128, G4], F16, tag="whh")
            nc.sync.dma_start(out=whh_sb, in_=whh_h[:, :])
            whh_neg = singles.tile([128, G4], F16, tag="whhn")
            nc.vector.tensor_scalar_mul(whh_neg, whh_sb, -1.0)
            bias_sb = singles.tile([1, G4], F32R, tag="bias")
            nc.sync.dma_start(out=bias_sb, in_=bias_h[:, :])

            # Warm-up matmuls: consume every weight tile once so later
            # matmuls inherit the weight-DMA dependencies via PE program
            # order instead of carrying their own sync waits (the LDW
            # instruction has very few wait slots).
            warm = psump.tile([128, 4, C], F32, tag="ps0")
            # DMA-independent dummy matmuls first: keep the PE busy through
            # the weight-DMA window so its p-state ramp (3us of continuous
            # execution -> full clock) completes before the first real xg
            # matmuls instead of running them at the cold clock.
            for _ in range(24):
                nc.tensor.matmul(warm[:, 0, :], lhsT=ones_sb[:, 0:128],
                                 rhs=ones_sb, start=True, stop=True,
                                 skip_group_check=True)
            nc.tensor.matmul(warm[:, 0, :], lhsT=whh_sb[:, 0:128],
                             rhs=whh_sb[:, 0:C], start=True, stop=True,
                             skip_group_check=True)
            nc.tensor.matmul(warm[:, 0, :], lhsT=whh_neg[:, 0:128],
                             rhs=whh_neg[:, 0:C], start=True, stop=True,
                             skip_group_check=True)
            nc.tensor.matmul(warm[:, 0, :], lhsT=wih_sb[:, 0, 0:128],
                             rhs=wih_sb[:, 1, 0:C], start=True, stop=True,
                             skip_group_check=True)
            nc.tensor.matmul(warm[:, 0, :], lhsT=bias_sb[:, 0:128],
                             rhs=ones_sb, start=True, stop=True,
                             skip_group_check=True)

            carry_h = []
            carry_c = []
            hs_pp = []
            for s in range(S):
                ch = singles.tile([128, BS], F16, tag=f"carryh{s}")
                cc = singles.tile([128, BS], F32, tag=f"carryc{s}")
                nc.vector.memset(ch, 0.0)
                nc.vector.memset(cc, 0.0)
                carry_h.append(ch)
                carry_c.append(cc)
                # persistent h ping-pong tiles: col 1+t = h at block t0+t,
                # col 0 is PERMANENTLY zero (memset once, never rewritten)
                # so the feedback matmuls can consume the full [0:L] range
                # with bank-aligned PSUM writes; the real carry term is
                # delivered separately by the tiny carry-column matmul.
                hsA = singles.tile([128, BS, L + 1], F16, tag=f"hsA{s}",
                                   name=f"hsA{s}")
                hsB = singles.tile([128, BS, L + 1], F16, tag=f"hsB{s}",
                                   name=f"hsB{s}")
                nc.vector.memset(hsA, 0.0)
                nc.vector.memset(hsB, 0.0)
                hs_pp.append((hsA, hsB))

            xt_r = xt_h[:, :, :].transpose([1, 0, 2]).rearrange(
                "p k (u t) -> p k u t", u=U)
            out_r = out_h[:, :].rearrange("p (u t) -> p u t", u=U)

            def gen_block(s, blk):
                """Generator: yields after the input phase and after each
                sweep so the emission driver can interleave streams."""
                u0 = s * BS
                t0 = blk * L
                # ---- x^T block in ----
                xt_t = work.tile([128, 2, BS, L], F32R, tag=f"xt{s}")
                for k in range(2):
                    nc.sync.dma_start(
                        out=xt_t[:, k],
                        in_=xt_r[:, k, u0:u0 + BS, t0:t0 + L],
                    )
                ps = psump.tile([128, 4, BS, L], F32, tag=f"ps{s}")
                # ---- xg = W_ih @ x + b  (per gate chunk, f32r matmuls) ----
                # start=True may only be set on the first matmul touching a
                # PSUM bank (it clears has_written for the whole bank).
                chunks_per_bank = max(1, 512 // C)
                for g in range(4):
                    for k in range(2):
                        nc.tensor.matmul(
                            ps[:, g],
                            lhsT=wih_sb[:, k, g * 128:(g + 1) * 128],
                            rhs=xt_t[:, k],
                            start=(k == 0 and g % chunks_per_bank == 0),
                            stop=False, skip_group_check=True,
                        )
                    nc.tensor.matmul(
                        ps[:, g],
                        lhsT=bias_sb[:, g * 128:(g + 1) * 128],
                        rhs=ones_sb,
                        start=False, stop=False, skip_group_check=True,
                    )
                # ---- carry column: gates(u, 0) += W_hh @ h_carry.  The
                # initial h guess is [carry | zeros], so this tiny strided
                # matmul is the whole sweep-0 feedback term.  The full-sweep
                # matmuls below see a permanent zero in hs col 0, so they
                # add nothing at the (u, 0) gate columns — exactly right,
                # since h at t0-1 is the exact carry, not an iterate.
                for g in range(4):
                    nc.tensor.matmul(
                        ps[:, g, :, 0],
                        lhsT=whh_sb[:, g * 128:(g + 1) * 128],
                        rhs=carry_h[s],
                        start=False, stop=False, skip_group_check=True,
                    )
                hsA, hsB = hs_pp[s]
                hs_prev = None      # h estimate from previous sweep
                hs_pprev = None     # ... from two sweeps ago
                yield

                for sw in range(K_SWEEPS):
                    last = sw == K_SWEEPS - 1
                    adt = F32 if last else F16
                    sfx = "32" if last else ""
                    # ---- gates(u, 1:) += W_hh @ (h_new - h_old)(u, :-1) ----
                    # The negative matmuls' operand (hs from two sweeps ago)
                    # is ready early, so they overlap the previous sweep's
                    # scan/tanh phase; only the positive matmuls sit on the
                    # critical path after the h update.  Sweep 0 has no
                    # feedback matmul at all (guess is zero past col 0) and
                    # sweep 1 no negative one.
                    if sw > 1:
                        for g in range(4):
                            nc.tensor.matmul(
                                ps[:, g],
                                lhsT=whh_neg[:, g * 128:(g + 1) * 128],
                                rhs=hs_pprev[:, :, 0:L],
                                start=False, stop=False,
                                skip_group_check=True,
                            )
                    if sw > 0:
                        for g in range(4):
                            nc.tensor.matmul(
                                ps[:, g],
                                lhsT=whh_sb[:, g * 128:(g + 1) * 128],
                                rhs=hs_prev[:, :, 0:L],
                                start=False, stop=(last and g == 3),
                                skip_group_check=True,
                            )
                    # ---- activations: one sigmoid over all 4 chunks;
                    # chunk 3 holds 2g so tanh(g) = 2*sigmoid(2g) - 1 ----
                    ifo = work.tile([128, 4, C], adt, tag=f"ifo{s}{sfx}",
                                    bufs=2 if last else 3)
                    nc.scalar.activation(out=ifo, in_=ps[:, :, :, :], func=sig)
                    # ---- z' = z/2 = (sig(2g) - 0.5) * sig(i); the halved
                    # c-recurrence c' = f*c' + z' keeps c' = c/2, recovered
                    # by tanh(2c') via the activation input scale ----
                    z = work.tile([128, C], adt, tag=f"z{s}{sfx}",
                                  bufs=2 if last else 4)
                    z_eng = nc.gpsimd if (Z_POOL and not last) else nc.vector
                    z_eng.scalar_tensor_tensor(
                        z, ifo[:, 3, :], -0.5, ifo[:, 0, :],
                        op0=add, op1=mult)
                    # ---- c'-recurrence scan per sequence ----
                    cfull = work.tile([128, C], F32, tag=f"c{s}", bufs=4)
                    for u in range(BS):
                        sc_eng = (nc.gpsimd if (SCAN_POOL and u % 2 == 1)
                                  else nc.vector)
                        sc_eng.tensor_tensor_scan(
                            out=cfull[:, u * L:(u + 1) * L],
                            data0=ifo[:, 1, u * L:(u + 1) * L],
                            data1=z[:, u * L:(u + 1) * L],
                            initial=carry_c[s][:, u:u + 1],
                            op0=mult, op1=add,
                        )
                    # ---- tanh(c) = tanh(2c') ----
                    # Final sweep: exact ScalarE tanh (sets output
                    # precision).  Intermediate sweeps: clamped odd-quintic
                    # approximation on the Vector engine, taking the
                    # instruction off the bottleneck ScalarE.
                    tcl = work.tile([128, C], adt, tag=f"tc{s}{sfx}",
                                    bufs=2 if last else 4)
                    if TANH_DVE and not last:
                        nc.vector._custom_dve(
                            TANH2C_OP, out=tcl, in0=cfull, in1=negone_sb,
                            s0=QA, s1=QB, imm2=QC)
                    else:
                        nc.scalar.activation(out=tcl, in_=cfull, func=tanh,
                                             scale=2.0)
                    # ---- h = sigmoid(o) * tanh(c) ----
                    o_v = ifo[:, 2, :].rearrange("p (u t) -> p u t", u=BS)
                    tc_v = tcl.rearrange("p (u t) -> p u t", u=BS)
                    if last:
                        # carries for next block first — the next block's
                        # carry-column matmul and scans depend on them, not
                        # on the full out tile.  The tiny h-carry multiply
                        # goes on DVE (has slack; Pool's in-order queue also
                        # holds the big out multiplies).
                        nc.vector.tensor_mul(
                            carry_h[s], o_v[:, :, L - 1], tc_v[:, :, L - 1])
                        nc.gpsimd.tensor_copy(
                            out=carry_c[s],
                            in_=cfull.rearrange("p (u t) -> p u t",
                                                u=BS)[:, :, L - 1])
                        out_t = work.tile([128, BS, L], F32, tag=f"out{s}",
                                          bufs=2)
                        o_eng = nc.gpsimd if OMUL_POOL else nc.vector
                        o_eng.tensor_mul(out_t, o_v, tc_v)
                        nc.sync.dma_start(
                            out=out_r[:, u0:u0 + BS, t0:t0 + L], in_=out_t,
                        )
                    else:
                        hs_next = hsB if hs_prev is hsA else hsA
                        h_eng = nc.gpsimd if HMUL_POOL else nc.vector
                        h_eng.tensor_mul(hs_next[:, :, 1:L + 1], o_v, tc_v)
                        hs_pprev = hs_prev
                        hs_prev = hs_next
                    yield

            for blk in range(NBLK):
                gens = [gen_block(s, blk) for s in range(S)]
                alive = list(gens)
                while alive:
                    for g in list(alive):
                        try:
                            next(g)
                        except StopIteration:
                            alive.remove(g)

    if not nc.is_finalized():
        nc.finalize()
    return nc


def _get_nc():
    if "nc" not in _NC_CACHE:
        _NC_CACHE["nc"] = _build_nc()
    return _NC_CACHE["nc"]


def _flip_padded(x, lengths):
    t = np.arange(x.shape[1])[None, :]
    Ln = lengths[:, None].astype(np.int64)
    idx = np.where(t < Ln, Ln - 1 - t, t)
    return np.take_along_axis(x, idx[:, :, None], axis=1)


def _pack_weights(W_ih, W_hh, b_ih, b_hh):
    # chunk order (i, f, o, g); the g chunk is pre-scaled by 2 because the
    # kernel computes tanh(g) as 2*sigmoid(2g) - 1 inside the fused sigmoid
    # instruction.
    Wi = W_ih.reshape(4, H, I)[PERM].copy()             # [4,128,256]
    Wi[3] *= 2.0
    wih = np.ascontiguousarray(
        Wi.transpose(2, 0, 1).reshape(2, 128, G4)).astype(np.float32)
    Wh = W_hh.reshape(4, H, H)[PERM].copy()             # [4,128,128]
    Wh[3] *= 2.0
    whh = np.ascontiguousarray(
        Wh.transpose(2, 0, 1).reshape(128, G4)).astype(np.float16)
    b4 = (b_ih + b_hh).reshape(4, H)[PERM].copy()
    b4[3] *= 2.0
    b = b4.reshape(1, G4).astype(np.float32)
    return wih, whh, np.ascontiguousarray(b)


def _pack_x(x_shard):
    # [U, T, I] -> [2, 128, U*T] with cols (u, t) u-major
    a = x_shard.transpose(2, 0, 1).reshape(2, 128, U * T)
    return np.ascontiguousarray(a).astype(np.float32)


def _run(inputs, trace=False):
    x = np.asarray(inputs["x"], np.float32)
    lengths = np.asarray(inputs["lengths"])
    Wf_ih = np.asarray(inputs["Wf_ih"], np.float32)
    Wf_hh = np.asarray(inputs["Wf_hh"], np.float32)
    bf_ih = np.asarray(inputs["bf_ih"], np.float32)
    bf_hh = np.asarray(inputs["bf_hh"], np.float32)
    Wb_ih = np.asarray(inputs["Wb_ih"], np.float32)
    Wb_hh = np.asarray(inputs["Wb_hh"], np.float32)
    bb_ih = np.asarray(inputs["bb_ih"], np.float32)
    bb_hh = np.asarray(inputs["bb_hh"], np.float32)

    x_rev = _flip_padded(x, lengths)
    wf = _pack_weights(Wf_ih, Wf_hh, bf_ih, bf_hh)
    wb = _pack_weights(Wb_ih, Wb_hh, bb_ih, bb_hh)

    in_maps = []
    for c in range(NCORES):
        if c < 4:
            xs = x[c * U:(c + 1) * U]
            wih, whh, b = wf
        else:
            xs = x_rev[(c - 4) * U:(c - 3) * U]
            wih, whh, b = wb
        in_maps.append({
            "xt": _pack_x(xs),
            "wih": wih,
            "whh": whh,
            "bias": b,
        })

    nc = _get_nc()
    res = run_bass_kernel_spmd(nc, in_maps, core_ids=list(range(NCORES)),
                               trace=trace)
    halves = []
    for c in range(NCORES):
        o = res.results[c]["out"].reshape(128, U, T).transpose(1, 2, 0)
        halves.append(o)
    fwd = np.concatenate(halves[0:4], axis=0)   # [32, T, 128]
    bwd = np.concatenate(halves[4:8], axis=0)   # [32, T, 128]
    out = np.concatenate([fwd, bwd], axis=-1).astype(np.float32)
    return out, res.exec_time_ns


def kernel(**inputs):
    out, _ = _run(inputs, trace=False)
    return out
